# revision 1
# baseline (speedup 1.0000x reference)
"""Trainium2 Bass kernel: CLUTRR-style GNN message passing (nn_CLUTRRV4).

Data-parallel across 8 NeuronCores (256 samples/core). Per core, samples are
packed 4-per-group (4 x 32 entity slots = 128 partitions). Entity states stay
SBUF-resident for all 8 message-passing steps; gather/scatter are expressed as
one-hot matmuls with the one-hot matrices generated on-chip from int16 index
tiles via is_equal. Matmuls run in float16 (full-rate PE + fast weight load, ~5e-4 matmul
accuracy); the state S is kept in fp32 master form with an fp16 shadow copy.
N=128 matmuls are widened to N=256 via step-0 duplicated rhs APs to stay in
the f32r full-rate regime.
"""
import sys
import numpy as np

if "/opt/trn_rl_repo" not in sys.path:
    sys.path.append("/opt/trn_rl_repo")

N_ENT, N_REL, D, E = 32, 20, 128, 64
N_STEPS = 8
N_CORES = 8
P = 128
GRP = 4  # samples per group


def _patch_ldw_opt():
    import os
    if os.environ.get("BASS_LDW_OPT") != "1":
        return
    from concourse import bass_utils as bu
    if getattr(bu, "_ldw_opt_patched", False):
        return
    orig = bu.run_command

    def run_command_ldw(cmd, *a, **kw):
        if isinstance(cmd, list):
            cmd = [c.replace("--enable-ldw-opt=false", "--enable-ldw-opt=true")
                   if isinstance(c, str) else c for c in cmd]
        return orig(cmd, *a, **kw)

    bu.run_command = run_command_ldw
    bu._ldw_opt_patched = True


def _build_nc(b_core, n_steps, use_gelu=True):
    from concourse import bacc, mybir
    from concourse.tile import TileContext
    from concourse.masks import make_identity

    f32 = mybir.dt.float32
    f32r = mybir.dt.float16
    i16 = mybir.dt.int16
    AF = mybir.ActivationFunctionType
    OP = mybir.AluOpType
    act_fn = AF.Gelu if use_gelu else AF.Identity

    G = b_core // GRP
    NPAIR = G // 2
    assert G % 4 == 0, "group count must be a multiple of 4 for rel/indeg packing"

    nc = bacc.Bacc()

    def din(name, shape, dtype=f32):
        return nc.declare_dram_parameter(name, list(shape), dtype, isOutput=False)

    d_s0 = din("s0", (P, G * P))
    d_gs = din("gsrc", (G, P, 256), i16)
    d_gt = din("gtgt", (G, P, 256), i16)
    d_gtc = din("gtc", (P, 2 * G), i16)
    d_rel = din("reloh", (G // 4, P, 256), f32r)
    d_rt4 = din("reltab4", (P, 256), f32r)
    d_ind = din("indeg", (G // 4, P, P), f32r)
    d_b2r = din("b2row", (P, P), f32r)
    d_qoh = din("qoh", (G, P, 8))
    d_w1ac = din("w1ac", (P, 512), f32r)
    d_w2m = din("w2m", (P, 256), f32r)
    d_w1u = din("w1u", (P, 512), f32r)
    d_w2u = din("w2u", (P, 256), f32r)
    d_b1u = din("b1u", (P, 2))
    d_b2u = din("b2u", (P, 1))
    d_cw1 = din("cw1", (P, 256))
    d_cb1 = din("cb1", (P, 1))
    d_cw2 = din("cw2", (P, 20))
    d_cb2 = din("cb2", (20, 1))
    d_out = nc.declare_dram_parameter("out", [20, b_core], f32, isOutput=True)

    with TileContext(nc) as tc:
        with (
            tc.tile_pool(name="c", bufs=1) as cp,
            tc.tile_pool(name="w", bufs=4) as wp,
            tc.tile_pool(name="pA", bufs=2, space="PSUM") as pA,
            tc.tile_pool(name="pH1", bufs=2, space="PSUM") as pH1,
            tc.tile_pool(name="pM", bufs=1, space="PSUM") as pM,
            tc.tile_pool(name="pG", bufs=1, space="PSUM") as pG,
            tc.tile_pool(name="pH3", bufs=1, space="PSUM") as pH3,
            tc.tile_pool(name="pS", bufs=1, space="PSUM") as pS,
        ):
            def cload(name, shape, dram, dtype=f32):
                t = cp.tile(list(shape), dtype, tag=name)
                nc.sync.dma_start(t[:], dram[:])
                return t

            w1ac = cload("w1ac", (P, 512), d_w1ac, f32r)
            w2m = cload("w2m", (P, 256), d_w2m, f32r)
            w1u = cload("w1u", (P, 512), d_w1u, f32r)
            w2u = cload("w2u", (P, 256), d_w2u, f32r)
            rt4 = cload("rt4", (P, 256), d_rt4, f32r)
            b2r = cload("b2r", (P, P), d_b2r, f32r)
            b1u = cload("b1u", (P, 2), d_b1u)
            b2u = cload("b2u", (P, 1), d_b2u)
            cw1 = cload("cw1", (P, 256), d_cw1)
            cb1 = cload("cb1", (P, 1), d_cb1)
            cw2 = cload("cw2", (P, 20), d_cw2)
            cb2 = cload("cb2", (20, 1), d_cb2)
            gtc = cload("gtc", (P, 2 * G), d_gtc, i16)

            eiota = cp.tile([P, 1], i16, tag="eiota")
            nc.gpsimd.iota(eiota[:], pattern=[[0, 1]], base=0, channel_multiplier=1)
            fiota = cp.tile([P, P], i16, tag="fiota")
            nc.gpsimd.iota(fiota[:], pattern=[[1, P]], base=0, channel_multiplier=0)
            ident = cp.tile([P, P], f32, tag="ident")
            make_identity(nc, ident[:])
            outsb = cp.tile([20, b_core], f32, tag="outsb")

            S, SR = [], []
            GS, GT, QOH = [None] * G, [None] * G, [None] * G
            RELP, INDP = [None] * (G // 4), [None] * (G // 4)
            for p in range(NPAIR):
                if p % 2 == 0:
                    j = p // 2
                    t = cp.tile([P, 256], f32r, tag=f"rp{j}")
                    nc.sync.dma_start(t[:], d_rel[j])
                    RELP[j] = t
                    t = cp.tile([P, P], f32r, tag=f"ip{j}")
                    nc.sync.dma_start(t[:], d_ind[j])
                    INDP[j] = t
                t = cp.tile([P, 256], f32, tag=f"S{p}")
                nc.sync.dma_start(t[:], d_s0[:, p * 256:(p + 1) * 256])
                S.append(t)
                t2 = cp.tile([P, 256], f32r, tag=f"Sr{p}")
                nc.gpsimd.tensor_copy(t2[:], t[:])
                SR.append(t2)
                for g in (2 * p, 2 * p + 1):
                    t = cp.tile([P, 256], i16, tag=f"gs{g}")
                    nc.sync.dma_start(t[:], d_gs[g])
                    GS[g] = t
                    t = cp.tile([P, 256], i16, tag=f"gt{g}")
                    nc.sync.dma_start(t[:], d_gt[g])
                    GT[g] = t
                    t = cp.tile([P, 8], f32, tag=f"q{g}")
                    nc.sync.dma_start(t[:], d_qoh[g])
                    QOH[g] = t

            mm = nc.tensor.matmul

            def dup2(ap_):
                """(K, n) AP -> (K, 2, n) with step-0 middle dim (rhs widening)."""
                k, n = ap_.shape
                return ap_[:, None, :].to_broadcast([k, 2, n])

            for t_step in range(n_steps):
                for p in range(NPAIR):
                    agg = pG.tile([P, 256], f32, tag="agg")
                    for gi in range(2):
                        g = 2 * p + gi
                        rb = (g % 4) * 32
                        # A = [S@W1a | S@W1c] in natural (slot-major) layout
                        aps = pA.tile([P, 512], f32, tag="aps")
                        mm(aps[:], lhsT=SR[p][:, gi * P:(gi + 1) * P], rhs=w1ac[:],
                           start=True, stop=True)
                        asb = wp.tile([P, 512], f32r, tag="asb")
                        nc.vector.tensor_copy(asb[:], aps[:])
                        # ent-major one-hots (DVE)
                        ohs = wp.tile([P, 256], f32r, tag="ohs")
                        nc.vector.tensor_tensor(
                            ohs[:], GS[g][:], eiota[:].to_broadcast([P, 256]),
                            op=OP.is_equal)
                        oht = wp.tile([P, 256], f32r, tag="oht")
                        nc.vector.tensor_tensor(
                            oht[:], GT[g][:], eiota[:].to_broadcast([P, 256]),
                            op=OP.is_equal)
                        # h1 = rel_bias + onehot_src@A + onehot_tgt@Bt (per featchunk)
                        h1 = pH1.tile([P, 512], f32, tag="h1")
                        for F in range(2):
                            o = h1[:, F * 256:(F + 1) * 256]
                            mm(o, lhsT=rt4[rb:rb + 20, F * P:(F + 1) * P],
                               rhs=RELP[g // 4][rb:rb + 20, :], start=True, stop=False,
                               tile_position=(rb, 0))
                            mm(o, lhsT=asb[:, F * P:(F + 1) * P], rhs=ohs[:],
                               start=False, stop=False)
                            mm(o, lhsT=asb[:, 256 + F * P:256 + (F + 1) * P],
                               rhs=oht[:], start=False, stop=True)
                        h1g = wp.tile([P, 512], f32r, tag="h1g")
                        nc.scalar.activation(h1g[:], h1[:], act_fn)
                        # msg layer 2, emitted edge-major
                        msg = pM.tile([P, 256], f32, tag="msg")
                        for ec in range(2):
                            o = msg[:, ec * P:(ec + 1) * P]
                            for F in range(2):
                                mm(o, lhsT=h1g[:, F * 256 + ec * P:F * 256 + (ec + 1) * P],
                                   rhs=w2m[:, F * P:(F + 1) * P],
                                   start=(F == 0), stop=(F == 1))
                        msb = wp.tile([P, 256], f32r, tag="msb")
                        nc.scalar.copy(msb[:], msg[:])
                        # edge-major masked tgt one-hot (DVE)
                        ohe = wp.tile([P, 256], f32r, tag="ohe")
                        for ec in range(2):
                            col = gtc[:, g * 2 + ec:g * 2 + ec + 1]
                            nc.vector.tensor_tensor(
                                ohe[:, ec * P:(ec + 1) * P],
                                col.to_broadcast([P, P]), fiota[:], op=OP.is_equal)
                        # scatter-add + msg_b2*indegree fold
                        o = agg[:, gi * P:(gi + 1) * P]
                        mm(o, lhsT=msb[:, 0:P], rhs=ohe[:, 0:P],
                           start=True, stop=False)
                        mm(o, lhsT=msb[:, P:256], rhs=ohe[:, P:256],
                           start=False, stop=False)
                        mm(o, lhsT=b2r[rb:rb + 1, :],
                           rhs=INDP[g // 4][rb:rb + 1, :],
                           start=False, stop=True, tile_position=(rb, 0))
                    # update MLP over the pair (256 slot cols)
                    gsb = wp.tile([P, 256], f32r, tag="gsb")
                    nc.scalar.copy(gsb[:], agg[:])
                    h3 = pH3.tile([P, 512], f32, tag="h3")
                    for mc in range(2):
                        o = h3[:, mc * 256:(mc + 1) * 256]
                        mm(o, lhsT=w1u[:, mc * P:(mc + 1) * P], rhs=SR[p][:],
                           start=True, stop=False)
                        mm(o, lhsT=w1u[:, 256 + mc * P:256 + (mc + 1) * P], rhs=gsb[:],
                           start=False, stop=True)
                    h3g = wp.tile([P, 512], f32r, tag="h3g")
                    for mc in range(2):
                        nc.scalar.activation(
                            h3g[:, mc * 256:(mc + 1) * 256],
                            h3[:, mc * 256:(mc + 1) * 256], act_fn,
                            bias=b1u[:, mc:mc + 1])
                    sn = pS.tile([P, 256], f32, tag="sn")
                    for kc in range(2):
                        mm(sn[:], lhsT=w2u[:, kc * P:(kc + 1) * P],
                           rhs=h3g[:, kc * 256:(kc + 1) * 256],
                           start=(kc == 0), stop=(kc == 1))
                    # S += sn + b2u (fp32 master), then refresh the f32r shadow
                    nc.vector.scalar_tensor_tensor(
                        out=S[p][:], in0=sn[:], scalar=b2u[:, 0:1], in1=S[p][:],
                        op0=OP.add, op1=OP.add)
                    nc.gpsimd.tensor_copy(SR[p][:], S[p][:])

            # classifier head (fp32 throughout; tiny)
            nbatch = (G + 15) // 16
            for bq in range(nbatch):
                jn = min(16, G - bq * 16)
                qps = pH1.tile([P, P], f32, tag="h1")
                for j in range(jn):
                    g = bq * 16 + j
                    p2, gi = divmod(g, 2)
                    stp = pA.tile([P, P], f32, tag="aps")
                    nc.tensor.transpose(stp[:], S[p2][:, gi * P:(gi + 1) * P], ident[:])
                    sts = wp.tile([P, P], f32, tag="sts")
                    nc.vector.tensor_copy(sts[:], stp[:])
                    mm(qps[:, j * 8:(j + 1) * 8], lhsT=sts[:], rhs=QOH[g][:],
                       start=True, stop=True)
                qcat = wp.tile([P, P], f32, tag="qcat")
                nc.vector.tensor_copy(qcat[:, 0:jn * 8], qps[:, 0:jn * 8])
                qv = qcat[:, 0:jn * 8].rearrange("p (g t f) -> p g t f", t=2, f=4)
                ncols = jn * 4
                hps = pM.tile([P, ncols], f32, tag="msg")
                mm(hps[:], lhsT=cw1[:, 0:P], rhs=qv[:, :, 0, :], start=True, stop=False)
                mm(hps[:], lhsT=cw1[:, P:256], rhs=qv[:, :, 1, :], start=False, stop=True)
                hg = wp.tile([P, ncols], f32, tag="hg")
                nc.scalar.activation(hg[:], hps[:], act_fn, bias=cb1[:, 0:1])
                ops_ = pG.tile([20, ncols], f32, tag="agg")
                mm(ops_[:], lhsT=cw2[:], rhs=hg[:], start=True, stop=True)
                nc.scalar.activation(
                    outsb[:, bq * 64:bq * 64 + ncols], ops_[:], AF.Identity,
                    bias=cb2[:, 0:1])
            nc.sync.dma_start(d_out[:], outsb[:])

    nc.finalize()
    return nc


def _host_prep_shared(inp, b_core):
    f = np.float32
    ee = np.asarray(inp["entity_embed"], f)
    w1 = np.asarray(inp["msg_W1"], f)
    reltab = np.asarray(inp["rel_embed"], f) @ w1[128:256] + np.asarray(inp["msg_b1"], f)
    rt4 = np.zeros((P, 256), f)
    b2r = np.zeros((P, P), f)
    for b in range(4):
        rt4[b * 32:b * 32 + 20] = reltab
        b2r[b * 32] = np.asarray(inp["msg_b2"], f)
    w2m_ = np.asarray(inp["msg_W2"], f)
    w1u_ = np.asarray(inp["upd_W1"], f)
    w2u_ = np.asarray(inp["upd_W2"], f)
    cw1_ = np.asarray(inp["cls_W1"], f)
    h = np.float16
    return {
        "s0": np.tile(ee.T, (1, b_core)).astype(f),
        "reltab4": rt4.astype(h),
        "b2row": b2r.astype(h),
        "w1ac": np.concatenate([w1[0:128], w1[256:384]], axis=1).astype(h),
        "w2m": np.concatenate([w2m_[0:128], w2m_[128:256]], axis=1).astype(h),
        "w1u": np.concatenate(
            [w1u_[0:128, 0:128], w1u_[0:128, 128:256],
             w1u_[128:256, 0:128], w1u_[128:256, 128:256]], axis=1).astype(h),
        "w2u": np.concatenate([w2u_[0:128], w2u_[128:256]], axis=1).astype(h),
        "b1u": np.asarray(inp["upd_b1"], f).reshape(2, 128).T.copy(),
        "b2u": np.asarray(inp["upd_b2"], f).reshape(128, 1).copy(),
        "cw1": np.concatenate([cw1_[0:128], cw1_[128:256]], axis=1).astype(f),
        "cb1": np.asarray(inp["cls_b1"], f).reshape(128, 1).copy(),
        "cw2": np.asarray(inp["cls_W2"], f).copy(),
        "cb2": np.asarray(inp["cls_b2"], f).reshape(20, 1).copy(),
    }


def _host_prep_core(inp, c, b_core):
    f = np.float32
    sl = slice(c * b_core, (c + 1) * b_core)
    src = np.asarray(inp["edge_src"])[sl].astype(np.int64)
    tgt = np.asarray(inp["edge_tgt"])[sl].astype(np.int64)
    rel = np.asarray(inp["edge_rel"])[sl].astype(np.int64)
    ne = np.asarray(inp["n_edges"])[sl].astype(np.int64)
    qs = np.asarray(inp["query_src"])[sl].astype(np.int64)
    qt = np.asarray(inp["query_tgt"])[sl].astype(np.int64)
    G = b_core // GRP

    mask = (np.arange(E)[None, :] < ne[:, None])
    soff = (np.arange(b_core) % GRP)[:, None] * 32
    gs = np.where(mask, soff + src, 255).astype(np.int16).reshape(G, 256)
    gt = np.where(mask, soff + tgt, 255).astype(np.int16).reshape(G, 256)

    relg = rel.reshape(G, 256)
    reloh = np.zeros((G // 4, P, 256), f)
    oh = (relg[:, None, :] == np.arange(20)[None, :, None]).astype(f)
    reloh.reshape(G // 4, 4, 32, 256)[:, :, :20] = oh.reshape(G // 4, 4, 20, 256)

    ind = np.zeros((b_core, 32), f)
    np.add.at(ind, (np.repeat(np.arange(b_core), E), tgt.ravel()),
              mask.ravel().astype(f))
    indp = np.zeros((G // 4, P, P), f)
    indp.reshape(G // 4, 4, 32, P)[:, :, 0, :] = ind.reshape(G // 4, 4, P)

    qoh = np.zeros((G, P, 8), f)
    s_all = np.arange(b_core)
    gidx = s_all // GRP
    sg = s_all % GRP
    qoh[gidx, sg * 32 + qs, sg] = 1.0
    qoh[gidx, sg * 32 + qt, 4 + sg] = 1.0

    return {
        "gsrc": np.ascontiguousarray(np.broadcast_to(gs[:, None, :], (G, P, 256))),
        "gtgt": np.ascontiguousarray(np.broadcast_to(gt[:, None, :], (G, P, 256))),
        "gtc": np.ascontiguousarray(gt.reshape(2 * G, P).T),
        "reloh": reloh.astype(np.float16),
        "indeg": indp.astype(np.float16),
        "qoh": qoh,
    }


_CACHE = {}


def kernel(**inputs):
    b = np.asarray(inputs["edge_src"]).shape[0]
    b_core = b // N_CORES
    _patch_ldw_opt()
    key = b_core
    if key not in _CACHE:
        _CACHE[key] = _build_nc(b_core, N_STEPS, use_gelu=True)
    nc = _CACHE[key]

    shared = _host_prep_shared(inputs, b_core)
    in_maps = []
    for c in range(N_CORES):
        m = dict(shared)
        m.update(_host_prep_core(inputs, c, b_core))
        in_maps.append(m)

    from concourse.bass_utils import run_bass_kernel_spmd
    res = run_bass_kernel_spmd(nc, in_maps, core_ids=list(range(N_CORES)))
    out = np.concatenate([r["out"].T for r in res.results], axis=0)
    return np.ascontiguousarray(out, dtype=np.float32)



# revision 10
# speedup vs baseline: 1.6491x; 1.6491x over previous
"""Trainium2 Bass kernel: CLUTRR-style GNN message passing (nn_CLUTRRV4).

Data-parallel across 8 NeuronCores (256 samples/core). Samples are bin-packed
4-per-group (4 x 32 entity slots = 128 partitions) such that each group's
total valid edge count is <= 128 (greedy + local-search on n_edges), so each
group has a single 128-wide edge block. Gather/scatter are one-hot matmuls
with host-precomputed fp8 one-hots. Per step:

  S8T = transpose(S8)                  # PE transpose, fp8
  xT  = S8T-gather(oh_src|oh_tgt)      # one fp8 matmul, N=256
  h1  = W1[src|tgt]-proj(xT) + reltab  # fp8 DoubleRow + K=20 DR blockdiag
  h1g = gelu(h1) -> fp8
  msg = h1g @ W2m                      # fp8 DoubleRow over hidden halves
  msb = msg + b2m (drain)              # padding edges masked via zero one-hots
  agg = scatter(msb, ohe)              # fp8 matmul
  h3  = W1u^T [S8|agg8]                # fp8 DoubleRow, 2-pair batched N=512
  h3g = gelu(h3 + b1u) -> fp8
  sn  = W2u^T h3g                      # fp8 DoubleRow
  S  += sn + b2u (fp32 master, DVE)    # then refresh fp8 shadow (gpsimd)

Classifier head gathers query states from the final S8T and runs in fp16.
The quantization error of S8 enters h1 only through a K=256 contraction
(project-after-gather), so fp8 errors average out (~1% final).
"""
import sys
import numpy as np

if "/opt/trn_rl_repo" not in sys.path:
    sys.path.append("/opt/trn_rl_repo")

import ml_dtypes

N_ENT, N_REL, D, E_IN = 32, 20, 128, 64
N_STEPS = 8
N_CORES = 8
P = 128
E = 128          # packed edges per group (hard cap, verified by packer)
GRP = 4          # samples per group
NP8 = ml_dtypes.float8_e4m3


def _build_nc(b_core, n_steps):
    from concourse import bacc, mybir
    from concourse.tile import TileContext

    f32 = mybir.dt.float32
    f16 = mybir.dt.float16
    f8 = mybir.dt.float8e4
    AF = mybir.ActivationFunctionType
    OP = mybir.AluOpType
    DRM = mybir.MatmulPerfMode.DoubleRow

    G = b_core // GRP          # 64 groups
    NQ = G // 4                # 16 quads (4 groups = 2 pairs each)

    nc = bacc.Bacc()

    def din(name, shape, dtype=f32):
        return nc.declare_dram_parameter(name, list(shape), dtype, isOutput=False)

    d_s0 = din("s0", (NQ, P, 512))
    d_u8i = din("u16init", (NQ, P, 512), f16)      # initial S16 halves of upd16
    d_oh2 = din("oh2", (G, P, 256), f16)           # [ohs | oht] slot-major
    d_ohe = din("ohe", (G, P, P), f8)              # edge-major tgt one-hot
    d_rbd = din("relbd", (G, N_REL, 1024), f8)     # blockdiag rel one-hot
    d_rt8 = din("rt8", (N_REL, 256), f8)           # reltab (incl. msg_b1)
    d_w1x = din("w1x16", (P, 512), f16)            # [W1a-F0|W1c-F0|W1a-F1|W1c-F1]
    d_w2m = din("w2m8", (P, 256), f8)              # [W2m rows 0:128 | 128:256]
    d_b2m = din("b2mB", (P, 512))                  # b2m replicated 128p x 4 blocks
    d_w1u = din("w1u16", (P, 512), f16)            # [S|G per mc]
    d_w2u = din("w2u16", (P, 256), f16)
    d_b1u = din("b1u", (P, 2))
    d_b2u = din("b2u", (P, 1))
    d_qoh = din("qoh16", (G, P, 8), f16)
    d_cw1s = din("cw1s", (P, P), f16)
    d_cw1t = din("cw1t", (P, P), f16)
    d_cb1 = din("cb1", (P, 1))
    d_cw2 = din("cw2", (P, N_REL), f16)
    d_cb2 = din("cb2", (N_REL, 1))
    d_id16 = din("id16", (P, P), f16)
    d_out = nc.declare_dram_parameter("out", [N_REL, b_core], f32, isOutput=True)

    with TileContext(nc) as tc:
        with (
            tc.tile_pool(name="c", bufs=1) as cp,
            tc.tile_pool(name="w", bufs=3) as wp,
            tc.tile_pool(name="pT", bufs=1, space="PSUM") as pT,
            tc.tile_pool(name="pX", bufs=2, space="PSUM") as pX,
            tc.tile_pool(name="pH1", bufs=2, space="PSUM") as pH1,
            tc.tile_pool(name="pM", bufs=1, space="PSUM") as pM,
            tc.tile_pool(name="pTl", bufs=2, space="PSUM") as pTl,
        ):
            def cload(name, shape, dram, dtype=f32):
                t = cp.tile(list(shape), dtype, tag=name)
                nc.sync.dma_start(t[:], dram[:])
                return t

            rt8 = cload("rt8", (N_REL, 256), d_rt8, f8)
            w1x = cload("w1x16", (P, 512), d_w1x, f16)
            w2m = cload("w2m8", (P, 256), d_w2m, f8)
            b2m = cload("b2mB", (P, 512), d_b2m)
            w1u = cload("w1u16", (P, 512), d_w1u, f16)
            w2u = cload("w2u16", (P, 256), d_w2u, f16)
            b1u = cload("b1u", (P, 2), d_b1u)
            b2u = cload("b2u", (P, 1), d_b2u)
            cw1s = cload("cw1s", (P, P), d_cw1s, f16)
            cw1t = cload("cw1t", (P, P), d_cw1t, f16)
            cb1 = cload("cb1", (P, 1), d_cb1)
            cw2 = cload("cw2", (P, N_REL), d_cw2, f16)
            cb2 = cload("cb2", (N_REL, 1), d_cb2)
            id16 = cload("id16", (P, P), d_id16, f16)
            outsb = cp.tile([N_REL, b_core], f32, tag="outsb")

            S2, U8 = [], []
            OH2, OHE, RBD, QOH = [None] * G, [None] * G, [None] * G, [None] * G
            for q in range(NQ):
                t = cp.tile([P, 512], f32, tag=f"S2_{q}")
                nc.sync.dma_start(t[:], d_s0[q])
                S2.append(t)
                t = cp.tile([P, 1024], f16, tag=f"U16_{q}", name=f"U16_{q}")
                nc.sync.dma_start(t[:, 0:512], d_u8i[q])
                U8.append(t)
                for gi in range(4):
                    g = q * 4 + gi
                    OH2[g] = cp.tile([P, 256], f16, tag=f"oh2_{g}",
                                     name=f"oh2_{g}")
                    nc.sync.dma_start(OH2[g][:], d_oh2[g])
                    OHE[g] = cp.tile([P, P], f8, tag=f"ohe_{g}",
                                     name=f"ohe_{g}")
                    nc.sync.dma_start(OHE[g][:], d_ohe[g])
                    RBD[g] = cp.tile([N_REL, 1024], f8, tag=f"rbd_{g}",
                                     name=f"rbd_{g}")
                    nc.sync.dma_start(RBD[g][:], d_rbd[g])
                    QOH[g] = cp.tile([P, 8], f16, tag=f"qoh_{g}",
                                     name=f"qoh_{g}")
                    nc.sync.dma_start(QOH[g][:], d_qoh[g])

            mm = nc.tensor.matmul

            def i2(ap):
                return ap.rearrange("p (i n) -> p i n", i=2)

            def head(q, S16Tsb):
                """Transpose + drain S16T for quad q's 4 groups into S16Tsb."""
                tps = pT.tile([P, 512], f16, tag="tps")
                for gi in range(4):
                    nc.tensor.transpose(
                        tps[:, gi * P:(gi + 1) * P],
                        U8[q][:, gi * P:(gi + 1) * P], id16[:])
                nc.scalar.copy(S16Tsb[:], tps[:])

            for t_step in range(n_steps):
                for q in range(NQ):
                    S16Tsb = wp.tile([P, 512], f16, tag="s16t")
                    head(q, S16Tsb)
                    h1g = wp.tile([P, 1024], f8, tag="h1g")
                    for h in range(2):      # two 2-group halves
                        xps = pX.tile([P, 512], f32, tag="xps")
                        for gi in (2 * h, 2 * h + 1):
                            g = q * 4 + gi
                            mm(xps[:, (gi % 2) * 256:(gi % 2) * 256 + 256],
                               lhsT=S16Tsb[:, gi * P:(gi + 1) * P],
                               rhs=OH2[g][:], start=True, stop=True)
                        x16 = wp.tile([P, 512], f16, tag="x16")
                        nc.vector.tensor_copy(x16[:], xps[:])
                        h1p = pH1.tile([P, 512], f32, tag="h1p")
                        for j in range(2):
                            g = q * 4 + 2 * h + j
                            o = h1p[:, j * 256:(j + 1) * 256]
                            mm(o, lhsT=i2(rt8[:]),
                               rhs=RBD[g][:].rearrange("p (i n) -> p i n", i=2)[:, :, 0:256],
                               start=True, stop=False, perf_mode=DRM)
                            for F in range(2):
                                for st in range(2):   # 0: src (W1a), 1: tgt (W1c)
                                    mm(o[:, F * P:(F + 1) * P],
                                       lhsT=w1x[:, (2 * F + st) * P:(2 * F + st + 1) * P],
                                       rhs=x16[:, j * 256 + st * P:j * 256 + (st + 1) * P],
                                       start=False, stop=(F == 1 and st == 1),
                                       skip_group_check=True)
                        nc.scalar.activation(
                            h1g[:, h * 512:(h + 1) * 512], h1p[:], AF.Gelu)
                    # msg layer 2 (edge-major out), 4 groups into one bank
                    mps = pM.tile([P, 512], f32, tag="mps")
                    for gi in range(4):
                        mm(mps[:, gi * P:(gi + 1) * P],
                           lhsT=h1g[:, gi * 256:(gi + 1) * 256].rearrange(
                               "p (i n) -> p i n", i=2),
                           rhs=i2(w2m[:]), start=True, stop=True, perf_mode=DRM)
                    msb = wp.tile([P, 512], f8, tag="msb")
                    nc.vector.tensor_tensor(msb[:], mps[:], b2m[:], op=OP.add)
                    # scatter into per-pair agg + update MLP (2-pair batched)
                    agg = pTl.tile([P, 512], f32, tag="tail")
                    for gi in range(4):
                        g = q * 4 + gi
                        mm(agg[:, gi * P:(gi + 1) * P],
                           lhsT=msb[:, gi * P:(gi + 1) * P],
                           rhs=OHE[g][:], start=True, stop=True)
                    nc.vector.tensor_copy(U8[q][:, 512:1024], agg[:])
                    h3g = wp.tile([P, 1024], f16, tag="h3g")
                    for mc in range(2):
                        h3p = pTl.tile([P, 512], f32, tag="tail")
                        mm(h3p[:], lhsT=w1u[:, mc * 256:mc * 256 + P],
                           rhs=U8[q][:, 0:512], start=True, stop=False)
                        mm(h3p[:], lhsT=w1u[:, mc * 256 + P:(mc + 1) * 256],
                           rhs=U8[q][:, 512:1024], start=False, stop=True)
                        nc.scalar.activation(
                            h3g[:, mc * 512:(mc + 1) * 512], h3p[:], AF.Gelu,
                            bias=b1u[:, mc:mc + 1])
                    snp = pTl.tile([P, 512], f32, tag="tail")
                    for mc in range(2):
                        mm(snp[:], lhsT=w2u[:, mc * P:(mc + 1) * P],
                           rhs=h3g[:, mc * 512:(mc + 1) * 512],
                           start=(mc == 0), stop=(mc == 1))
                    nc.vector.scalar_tensor_tensor(
                        out=S2[q][:], in0=snp[:], scalar=b2u[:, 0:1],
                        in1=S2[q][:], op0=OP.add, op1=OP.add)
                    nc.gpsimd.tensor_copy(U8[q][:, 0:512], S2[q][:])

            # classifier head: gather query states from final S8T, fp16 MLP
            for q in range(NQ):
                S16Tsb = wp.tile([P, 512], f16, tag="s16t")
                head(q, S16Tsb)
                qps = pX.tile([P, 512], f32, tag="xps")
                for gi in range(4):
                    g = q * 4 + gi
                    mm(qps[:, gi * 8:(gi + 1) * 8],
                       lhsT=S16Tsb[:, gi * P:(gi + 1) * P],
                       rhs=QOH[g][:], start=True, stop=True)
                qt16 = wp.tile([P, 32], f16, tag="qt16")
                nc.vector.tensor_copy(qt16[:], qps[:, 0:32])
                qv = qt16[:].rearrange("p (g t f) -> p g t f", t=2, f=4)
                hcp = pH1.tile([P, 512], f32, tag="h1p")
                mm(hcp[:, 0:16], lhsT=cw1s[:], rhs=qv[:, :, 0, :],
                   start=True, stop=False)
                mm(hcp[:, 0:16], lhsT=cw1t[:], rhs=qv[:, :, 1, :],
                   start=False, stop=True)
                hcg = wp.tile([P, 16], f16, tag="hcg")
                nc.scalar.activation(hcg[:], hcp[:, 0:16], AF.Gelu,
                                     bias=cb1[:, 0:1])
                lps = pM.tile([P, 512], f32, tag="mps")
                mm(lps[0:N_REL, 0:16], lhsT=cw2[:], rhs=hcg[:],
                   start=True, stop=True)
                nc.scalar.activation(
                    outsb[:, q * 16:(q + 1) * 16], lps[0:N_REL, 0:16],
                    AF.Identity, bias=cb2[:, 0:1])
            nc.sync.dma_start(d_out[:], outsb[:])

    nc.finalize()
    return nc


def _pack_groups(n_edges):
    """Pack 2048 samples into 512 groups of 4 with per-group edge sum <= 128.
    Returns list of 512 lists of 4 sample indices. Falls back to max found."""
    import heapq, random
    ne = np.asarray(n_edges, dtype=np.int64)
    B = ne.shape[0]
    G = B // GRP
    order = np.argsort(-ne)
    gsum = np.zeros(G, np.int64)
    gcnt = np.zeros(G, np.int64)
    assign = [[] for _ in range(G)]
    heap = [(0, 0, g) for g in range(G)]
    heapq.heapify(heap)
    for idx in order:
        while True:
            s, c, g = heapq.heappop(heap)
            if gcnt[g] == c and gsum[g] == s:
                break
        assign[g].append(int(idx))
        gsum[g] += ne[idx]
        gcnt[g] += 1
        if gcnt[g] < GRP:
            heapq.heappush(heap, (gsum[g], gcnt[g], g))
    rng = random.Random(0)
    for _ in range(300000):
        over = [g for g in range(G) if gsum[g] > E]
        if not over:
            break
        g1 = rng.choice(over)
        found = False
        idxs1 = sorted(assign[g1], key=lambda i: -ne[i])
        g2s = list(range(G))
        rng.shuffle(g2s)
        for a in idxs1:
            na = ne[a]
            for g2 in g2s:
                if g2 == g1:
                    continue
                for b in assign[g2]:
                    nb = ne[b]
                    if nb < na and gsum[g2] + na - nb <= E and \
                            gsum[g1] - na + nb < gsum[g1]:
                        assign[g1].remove(a); assign[g2].remove(b)
                        assign[g1].append(b); assign[g2].append(a)
                        gsum[g1] += nb - na; gsum[g2] += na - nb
                        found = True
                        break
                if found:
                    break
            if found:
                break
        if not found:
            for _ in range(4000):
                g2 = rng.randrange(G)
                if g2 == g1:
                    continue
                a = rng.choice(assign[g1]); b = rng.choice(assign[g2])
                na, nb = ne[a], ne[b]
                n1 = gsum[g1] - na + nb
                n2 = gsum[g2] - nb + na
                if n1 <= gsum[g1] and n2 <= max(E, gsum[g2]) and \
                        not (n1 == gsum[g1] and n2 == gsum[g2]):
                    assign[g1].remove(a); assign[g2].remove(b)
                    assign[g1].append(b); assign[g2].append(a)
                    gsum[g1] = n1; gsum[g2] = n2
                    break
    assert gsum.max() <= E, f"edge packing failed: max group sum {gsum.max()}"
    return assign


def _host_prep_shared(inp):
    f = np.float32
    w1 = np.asarray(inp["msg_W1"], f)
    reltab = np.asarray(inp["rel_embed"], f) @ w1[128:256] + \
        np.asarray(inp["msg_b1"], f)                      # (20, 256)
    W1a, W1c = w1[0:128], w1[256:384]                     # (128, 256) each
    w1x = np.concatenate(
        [W1a[:, 0:128], W1c[:, 0:128], W1a[:, 128:256], W1c[:, 128:256]],
        axis=1)                                           # (128, 512)
    w2m_ = np.asarray(inp["msg_W2"], f)                   # (256, 128)
    w2m8 = np.concatenate([w2m_[0:128], w2m_[128:256]], axis=1)  # (128, 256)
    b2m = np.asarray(inp["msg_b2"], f)                    # (128,)
    b2mB = np.tile(b2m[None, :], (P, 4)).astype(f)        # (128, 512)
    u1 = np.asarray(inp["upd_W1"], f)                     # (256, 256)
    w1u8 = np.concatenate(
        [u1[0:128, 0:128], u1[128:256, 0:128],
         u1[0:128, 128:256], u1[128:256, 128:256]], axis=1)  # (128, 512)
    u2 = np.asarray(inp["upd_W2"], f)                     # (256, 128)
    w2u8 = np.concatenate([u2[0:128], u2[128:256]], axis=1)
    cw1 = np.asarray(inp["cls_W1"], f)                    # (256, 128)
    ee = np.asarray(inp["entity_embed"], f)               # (32, 128)
    s0q = np.tile(ee.T, (1, 16))                          # (128, 512) per quad
    return {
        "rt8": reltab.astype(NP8),
        "w1x16": w1x.astype(np.float16),
        "w2m8": w2m8.astype(NP8),
        "b2mB": b2mB,
        "w1u16": w1u8.astype(np.float16),
        "w2u16": w2u8.astype(np.float16),
        "b1u": np.asarray(inp["upd_b1"], f).reshape(2, 128).T.copy(),
        "b2u": np.asarray(inp["upd_b2"], f).reshape(128, 1).copy(),
        "cw1s": cw1[0:128].astype(np.float16),
        "cw1t": cw1[128:256].astype(np.float16),
        "cb1": np.asarray(inp["cls_b1"], f).reshape(128, 1).copy(),
        "cw2": np.asarray(inp["cls_W2"], f).astype(np.float16),
        "cb2": np.asarray(inp["cls_b2"], f).reshape(20, 1).copy(),
        "id16": np.eye(P, dtype=np.float16),
        "_s0q": s0q.astype(f),
    }


def _host_prep_core(inp, shared, groups):
    """groups: list of 64 lists of 4 global sample indices."""
    f = np.float32
    G = len(groups)
    NQ = G // 4
    src = np.asarray(inp["edge_src"]).astype(np.int64)
    tgt = np.asarray(inp["edge_tgt"]).astype(np.int64)
    rel = np.asarray(inp["edge_rel"]).astype(np.int64)
    ne = np.asarray(inp["n_edges"]).astype(np.int64)
    qs = np.asarray(inp["query_src"]).astype(np.int64)
    qt = np.asarray(inp["query_tgt"]).astype(np.int64)

    oh2 = np.zeros((G, P, 256), np.float16)
    ohe = np.zeros((G, P, P), NP8)
    rbd = np.zeros((G, N_REL, 1024), NP8)
    qoh = np.zeros((G, P, 8), np.float16)
    for g, samples in enumerate(groups):
        col = 0
        for j, sid in enumerate(samples):
            n = int(ne[sid])
            if n > 0:
                ss = src[sid, :n] + 32 * j
                tt = tgt[sid, :n] + 32 * j
                rr = rel[sid, :n]
                cols = np.arange(col, col + n)
                oh2[g, ss, cols] = 1.0
                oh2[g, tt, 128 + cols] = 1.0
                ohe[g, cols, tt] = 1.0
                rbd[g, rr, cols] = 1.0
                rbd[g, rr, 640 + cols] = 1.0
                col += n
            qoh[g, 32 * j + qs[sid], j] = 1.0
            qoh[g, 32 * j + qt[sid], 4 + j] = 1.0

    s0 = np.broadcast_to(shared["_s0q"][None], (NQ, P, 512))
    return {
        "s0": np.ascontiguousarray(s0),
        "u16init": np.ascontiguousarray(s0).astype(np.float16),
        "oh2": oh2,
        "ohe": ohe,
        "relbd": rbd,
        "qoh16": qoh,
    }


_CACHE = {}


def kernel(**inputs):
    b = np.asarray(inputs["edge_src"]).shape[0]
    b_core = b // N_CORES
    key = b_core
    if key not in _CACHE:
        _CACHE[key] = _build_nc(b_core, N_STEPS)
    nc = _CACHE[key]

    assign = _pack_groups(inputs["n_edges"])
    shared = _host_prep_shared(inputs)
    pub_shared = {k: v for k, v in shared.items() if not k.startswith("_")}
    G_core = b_core // GRP
    in_maps = []
    for c in range(N_CORES):
        groups = assign[c * G_core:(c + 1) * G_core]
        m = dict(pub_shared)
        m.update(_host_prep_core(inputs, shared, groups))
        in_maps.append(m)

    from concourse.bass_utils import run_bass_kernel_spmd
    res = run_bass_kernel_spmd(nc, in_maps, core_ids=list(range(N_CORES)))

    out = np.empty((b, N_REL), dtype=np.float32)
    for c in range(N_CORES):
        oc = res.results[c]["out"]     # (20, b_core)
        groups = assign[c * G_core:(c + 1) * G_core]
        for g, samples in enumerate(groups):
            for j, sid in enumerate(samples):
                out[sid] = oc[:, g * 4 + j]
    return out


# revision 12
# speedup vs baseline: 1.9221x; 1.1655x over previous
"""Trainium2 Bass kernel: CLUTRR-style GNN message passing (nn_CLUTRRV4).

Data-parallel across 8 NeuronCores (256 samples/core). Samples are bin-packed
4-per-group (4 x 32 entity slots = 128 partitions) such that each group's
total valid edge count is <= 128 (greedy + local-search on n_edges), so each
group has a single 128-wide edge block. Gather/scatter are one-hot matmuls
with host-precomputed fp8 one-hots. Per step:

  S8T = transpose(S8)                  # PE transpose, fp8
  xT  = S8T-gather(oh_src|oh_tgt)      # one fp8 matmul, N=256
  h1  = W1[src|tgt]-proj(xT) + reltab  # fp8 DoubleRow + K=20 DR blockdiag
  h1g = gelu(h1) -> fp8
  msg = h1g @ W2m                      # fp8 DoubleRow over hidden halves
  msb = msg + b2m (drain)              # padding edges masked via zero one-hots
  agg = scatter(msb, ohe)              # fp8 matmul
  h3  = W1u^T [S8|agg8]                # fp8 DoubleRow, 2-pair batched N=512
  h3g = gelu(h3 + b1u) -> fp8
  sn  = W2u^T h3g                      # fp8 DoubleRow
  S  += sn + b2u (fp32 master, DVE)    # then refresh fp8 shadow (gpsimd)

Classifier head gathers query states from the final S8T and runs in fp16.
The quantization error of S8 enters h1 only through a K=256 contraction
(project-after-gather), so fp8 errors average out (~1% final).
"""
import sys
import numpy as np

if "/opt/trn_rl_repo" not in sys.path:
    sys.path.append("/opt/trn_rl_repo")

import ml_dtypes

N_ENT, N_REL, D, E_IN = 32, 20, 128, 64
N_STEPS = 8
N_CORES = 8
P = 128
E = 128          # packed edges per group (hard cap, verified by packer)
GRP = 4          # samples per group
NP8 = ml_dtypes.float8_e4m3


def _build_nc(b_core, n_steps):
    from concourse import bacc, mybir
    from concourse.tile import TileContext

    f32 = mybir.dt.float32
    f16 = mybir.dt.float16
    f8 = mybir.dt.float8e4
    AF = mybir.ActivationFunctionType
    OP = mybir.AluOpType
    DRM = mybir.MatmulPerfMode.DoubleRow

    G = b_core // GRP          # 64 groups
    NQ = G // 4                # 16 quads (4 groups = 2 pairs each)

    nc = bacc.Bacc()

    def din(name, shape, dtype=f32):
        return nc.declare_dram_parameter(name, list(shape), dtype, isOutput=False)

    d_s0 = din("s0", (NQ, P, 512))
    d_u8i = din("u16init", (NQ, P, 512), f16)      # initial S16 halves of upd16
    d_oh2 = din("oh2", (G, P, 256), f16)           # [ohs | oht] slot-major
    d_ohe = din("ohe", (G, P, P), f8)              # edge-major tgt one-hot
    d_relq = din("relq", (NQ, N_REL, 1024), f8)    # per-quad rel one-hot [F0 | F1]
    d_rt8 = din("rt8", (N_REL, 256), f8)           # reltab (incl. msg_b1)
    d_w1x = din("w1x16", (P, 512), f16)            # [W1a-F0|W1c-F0|W1a-F1|W1c-F1]
    d_w2m = din("w2m8", (P, 256), f8)              # [W2m rows 0:128 | 128:256]
    d_b2m = din("b2mB", (P, 512))                  # b2m replicated 128p x 4 blocks
    d_w1u = din("w1u16", (P, 512), f16)            # [S|G per mc]
    d_w2u = din("w2u16", (P, 256), f16)
    d_b1u = din("b1u", (P, 2))
    d_b2u = din("b2u", (P, 1))
    d_qoh = din("qoh16", (G, P, 8), f16)
    d_cw1s = din("cw1s", (P, P), f16)
    d_cw1t = din("cw1t", (P, P), f16)
    d_cb1 = din("cb1", (P, 1))
    d_cw2 = din("cw2", (P, N_REL), f16)
    d_cb2 = din("cb2", (N_REL, 1))
    d_id16 = din("id16", (P, P), f16)
    d_out = nc.declare_dram_parameter("out", [N_REL, b_core], f32, isOutput=True)

    with TileContext(nc) as tc:
        with (
            tc.tile_pool(name="c", bufs=1) as cp,
            tc.tile_pool(name="w", bufs=3) as wp,
            tc.tile_pool(name="pT", bufs=1, space="PSUM") as pT,
            tc.tile_pool(name="pX", bufs=2, space="PSUM") as pX,
            tc.tile_pool(name="pH1", bufs=2, space="PSUM") as pH1,
            tc.tile_pool(name="pM", bufs=1, space="PSUM") as pM,
            tc.tile_pool(name="pTl", bufs=2, space="PSUM") as pTl,
        ):
            def cload(name, shape, dram, dtype=f32):
                t = cp.tile(list(shape), dtype, tag=name)
                nc.sync.dma_start(t[:], dram[:])
                return t

            rt8 = cload("rt8", (N_REL, 256), d_rt8, f8)
            w1x = cload("w1x16", (P, 512), d_w1x, f16)
            w2m = cload("w2m8", (P, 256), d_w2m, f8)
            b2m = cload("b2mB", (P, 512), d_b2m)
            w1u = cload("w1u16", (P, 512), d_w1u, f16)
            w2u = cload("w2u16", (P, 256), d_w2u, f16)
            b1u = cload("b1u", (P, 2), d_b1u)
            b2u = cload("b2u", (P, 1), d_b2u)
            cw1s = cload("cw1s", (P, P), d_cw1s, f16)
            cw1t = cload("cw1t", (P, P), d_cw1t, f16)
            cb1 = cload("cb1", (P, 1), d_cb1)
            cw2 = cload("cw2", (P, N_REL), d_cw2, f16)
            cb2 = cload("cb2", (N_REL, 1), d_cb2)
            id16 = cload("id16", (P, P), d_id16, f16)
            outsb = cp.tile([N_REL, b_core], f32, tag="outsb")

            S2, U8 = [], []
            RELQ = [None] * NQ
            OH2, OHE, QOH = [None] * G, [None] * G, [None] * G
            for q in range(NQ):
                RELQ[q] = cp.tile([N_REL, 1024], f8, tag=f"relq_{q}",
                                  name=f"relq_{q}")
                nc.sync.dma_start(RELQ[q][:], d_relq[q])
                t = cp.tile([P, 512], f32, tag=f"S2_{q}")
                nc.sync.dma_start(t[:], d_s0[q])
                S2.append(t)
                t = cp.tile([P, 1024], f16, tag=f"U16_{q}", name=f"U16_{q}")
                nc.sync.dma_start(t[:, 0:512], d_u8i[q])
                U8.append(t)
                for gi in range(4):
                    g = q * 4 + gi
                    OH2[g] = cp.tile([P, 256], f16, tag=f"oh2_{g}",
                                     name=f"oh2_{g}")
                    nc.sync.dma_start(OH2[g][:], d_oh2[g])
                    OHE[g] = cp.tile([P, P], f8, tag=f"ohe_{g}",
                                     name=f"ohe_{g}")
                    nc.sync.dma_start(OHE[g][:], d_ohe[g])

                    QOH[g] = cp.tile([P, 8], f16, tag=f"qoh_{g}",
                                     name=f"qoh_{g}")
                    nc.sync.dma_start(QOH[g][:], d_qoh[g])

            mm = nc.tensor.matmul

            def i2(ap):
                return ap.rearrange("p (i n) -> p i n", i=2)

            def head(q, S16Tsb):
                """Transpose + drain S16T for quad q's 4 groups into S16Tsb."""
                tps = pT.tile([P, 512], f16, tag="tps")
                for gi in range(4):
                    nc.tensor.transpose(
                        tps[:, gi * P:(gi + 1) * P],
                        U8[q][:, gi * P:(gi + 1) * P], id16[:])
                nc.scalar.copy(S16Tsb[:], tps[:])

            for t_step in range(n_steps):
                for q in range(NQ):
                    S16Tsb = wp.tile([P, 512], f16, tag="s16t")
                    head(q, S16Tsb)
                    x16 = wp.tile([P, 1024], f16, tag="x16")
                    for h in range(2):      # two 2-group halves
                        xps = pX.tile([P, 512], f32, tag="xps")
                        for gi in (2 * h, 2 * h + 1):
                            g = q * 4 + gi
                            mm(xps[:, (gi % 2) * 256:(gi % 2) * 256 + 256],
                               lhsT=S16Tsb[:, gi * P:(gi + 1) * P],
                               rhs=OH2[g][:], start=True, stop=True)
                        nc.vector.tensor_copy(x16[:, h * 512:(h + 1) * 512],
                                              xps[:])
                    # h1 per F-chunk, all 4 groups batched (N=512)
                    h1g = wp.tile([P, 1024], f8, tag="h1g")
                    xv = x16[:].rearrange("p (g st e) -> p g st e", st=2, e=P)
                    for F in range(2):
                        h1p = pH1.tile([P, 512], f32, tag="h1p")
                        mm(h1p[:], lhsT=rt8[:, F * P:(F + 1) * P],
                           rhs=RELQ[q][:, F * 512:(F + 1) * 512],
                           start=True, stop=False)
                        for st in range(2):   # 0: src (W1a), 1: tgt (W1c)
                            mm(h1p[:],
                               lhsT=w1x[:, (2 * F + st) * P:(2 * F + st + 1) * P],
                               rhs=xv[:, :, st, :],
                               start=False, stop=(st == 1))
                        nc.scalar.activation(
                            h1g[:, F * 512:(F + 1) * 512], h1p[:], AF.Gelu)
                    # msg layer 2 (edge-major out), 4 groups into one bank
                    mps = pM.tile([P, 512], f32, tag="mps")
                    h1v = h1g[:].rearrange("p (i g e) -> p i g e", i=2, e=P)
                    for gi in range(4):
                        mm(mps[:, gi * P:(gi + 1) * P],
                           lhsT=h1v[:, :, gi, :],
                           rhs=i2(w2m[:]), start=True, stop=True, perf_mode=DRM)
                    msb = wp.tile([P, 512], f8, tag="msb")
                    nc.vector.tensor_tensor(msb[:], mps[:], b2m[:], op=OP.add)
                    # scatter into per-pair agg + update MLP (2-pair batched)
                    agg = pTl.tile([P, 512], f32, tag="tail")
                    for gi in range(4):
                        g = q * 4 + gi
                        mm(agg[:, gi * P:(gi + 1) * P],
                           lhsT=msb[:, gi * P:(gi + 1) * P],
                           rhs=OHE[g][:], start=True, stop=True)
                    nc.vector.tensor_copy(U8[q][:, 512:1024], agg[:])
                    h3g = wp.tile([P, 1024], f16, tag="h3g")
                    for mc in range(2):
                        h3p = pTl.tile([P, 512], f32, tag="tail")
                        mm(h3p[:], lhsT=w1u[:, mc * 256:mc * 256 + P],
                           rhs=U8[q][:, 0:512], start=True, stop=False)
                        mm(h3p[:], lhsT=w1u[:, mc * 256 + P:(mc + 1) * 256],
                           rhs=U8[q][:, 512:1024], start=False, stop=True)
                        nc.scalar.activation(
                            h3g[:, mc * 512:(mc + 1) * 512], h3p[:], AF.Gelu,
                            bias=b1u[:, mc:mc + 1])
                    snp = pTl.tile([P, 512], f32, tag="tail")
                    for mc in range(2):
                        mm(snp[:], lhsT=w2u[:, mc * P:(mc + 1) * P],
                           rhs=h3g[:, mc * 512:(mc + 1) * 512],
                           start=(mc == 0), stop=(mc == 1))
                    nc.vector.scalar_tensor_tensor(
                        out=S2[q][:], in0=snp[:], scalar=b2u[:, 0:1],
                        in1=S2[q][:], op0=OP.add, op1=OP.add)
                    nc.gpsimd.tensor_copy(U8[q][:, 0:512], S2[q][:])

            # classifier head: gather query states from final S8T, fp16 MLP
            for q in range(NQ):
                S16Tsb = wp.tile([P, 512], f16, tag="s16t")
                head(q, S16Tsb)
                qps = pX.tile([P, 512], f32, tag="xps")
                for gi in range(4):
                    g = q * 4 + gi
                    mm(qps[:, gi * 8:(gi + 1) * 8],
                       lhsT=S16Tsb[:, gi * P:(gi + 1) * P],
                       rhs=QOH[g][:], start=True, stop=True)
                qt16 = wp.tile([P, 32], f16, tag="qt16")
                nc.vector.tensor_copy(qt16[:], qps[:, 0:32])
                qv = qt16[:].rearrange("p (g t f) -> p g t f", t=2, f=4)
                hcp = pH1.tile([P, 512], f32, tag="h1p")
                mm(hcp[:, 0:16], lhsT=cw1s[:], rhs=qv[:, :, 0, :],
                   start=True, stop=False)
                mm(hcp[:, 0:16], lhsT=cw1t[:], rhs=qv[:, :, 1, :],
                   start=False, stop=True)
                hcg = wp.tile([P, 16], f16, tag="hcg")
                nc.scalar.activation(hcg[:], hcp[:, 0:16], AF.Gelu,
                                     bias=cb1[:, 0:1])
                lps = pM.tile([P, 512], f32, tag="mps")
                mm(lps[0:N_REL, 0:16], lhsT=cw2[:], rhs=hcg[:],
                   start=True, stop=True)
                nc.scalar.activation(
                    outsb[:, q * 16:(q + 1) * 16], lps[0:N_REL, 0:16],
                    AF.Identity, bias=cb2[:, 0:1])
            nc.sync.dma_start(d_out[:], outsb[:])

    nc.finalize()
    return nc


def _pack_groups(n_edges):
    """Pack 2048 samples into 512 groups of 4 with per-group edge sum <= 128.
    Returns list of 512 lists of 4 sample indices. Falls back to max found."""
    import heapq, random
    ne = np.asarray(n_edges, dtype=np.int64)
    B = ne.shape[0]
    G = B // GRP
    order = np.argsort(-ne)
    gsum = np.zeros(G, np.int64)
    gcnt = np.zeros(G, np.int64)
    assign = [[] for _ in range(G)]
    heap = [(0, 0, g) for g in range(G)]
    heapq.heapify(heap)
    for idx in order:
        while True:
            s, c, g = heapq.heappop(heap)
            if gcnt[g] == c and gsum[g] == s:
                break
        assign[g].append(int(idx))
        gsum[g] += ne[idx]
        gcnt[g] += 1
        if gcnt[g] < GRP:
            heapq.heappush(heap, (gsum[g], gcnt[g], g))
    rng = random.Random(0)
    for _ in range(300000):
        over = [g for g in range(G) if gsum[g] > E]
        if not over:
            break
        g1 = rng.choice(over)
        found = False
        idxs1 = sorted(assign[g1], key=lambda i: -ne[i])
        g2s = list(range(G))
        rng.shuffle(g2s)
        for a in idxs1:
            na = ne[a]
            for g2 in g2s:
                if g2 == g1:
                    continue
                for b in assign[g2]:
                    nb = ne[b]
                    if nb < na and gsum[g2] + na - nb <= E and \
                            gsum[g1] - na + nb < gsum[g1]:
                        assign[g1].remove(a); assign[g2].remove(b)
                        assign[g1].append(b); assign[g2].append(a)
                        gsum[g1] += nb - na; gsum[g2] += na - nb
                        found = True
                        break
                if found:
                    break
            if found:
                break
        if not found:
            for _ in range(4000):
                g2 = rng.randrange(G)
                if g2 == g1:
                    continue
                a = rng.choice(assign[g1]); b = rng.choice(assign[g2])
                na, nb = ne[a], ne[b]
                n1 = gsum[g1] - na + nb
                n2 = gsum[g2] - nb + na
                if n1 <= gsum[g1] and n2 <= max(E, gsum[g2]) and \
                        not (n1 == gsum[g1] and n2 == gsum[g2]):
                    assign[g1].remove(a); assign[g2].remove(b)
                    assign[g1].append(b); assign[g2].append(a)
                    gsum[g1] = n1; gsum[g2] = n2
                    break
    assert gsum.max() <= E, f"edge packing failed: max group sum {gsum.max()}"
    return assign


def _host_prep_shared(inp):
    f = np.float32
    w1 = np.asarray(inp["msg_W1"], f)
    reltab = np.asarray(inp["rel_embed"], f) @ w1[128:256] + \
        np.asarray(inp["msg_b1"], f)                      # (20, 256)
    W1a, W1c = w1[0:128], w1[256:384]                     # (128, 256) each
    w1x = np.concatenate(
        [W1a[:, 0:128], W1c[:, 0:128], W1a[:, 128:256], W1c[:, 128:256]],
        axis=1)                                           # (128, 512)
    w2m_ = np.asarray(inp["msg_W2"], f)                   # (256, 128)
    w2m8 = np.concatenate([w2m_[0:128], w2m_[128:256]], axis=1)  # (128, 256)
    b2m = np.asarray(inp["msg_b2"], f)                    # (128,)
    b2mB = np.tile(b2m[None, :], (P, 4)).astype(f)        # (128, 512)
    u1 = np.asarray(inp["upd_W1"], f)                     # (256, 256)
    w1u8 = np.concatenate(
        [u1[0:128, 0:128], u1[128:256, 0:128],
         u1[0:128, 128:256], u1[128:256, 128:256]], axis=1)  # (128, 512)
    u2 = np.asarray(inp["upd_W2"], f)                     # (256, 128)
    w2u8 = np.concatenate([u2[0:128], u2[128:256]], axis=1)
    cw1 = np.asarray(inp["cls_W1"], f)                    # (256, 128)
    ee = np.asarray(inp["entity_embed"], f)               # (32, 128)
    s0q = np.tile(ee.T, (1, 16))                          # (128, 512) per quad
    return {
        "rt8": reltab.astype(NP8),
        "w1x16": w1x.astype(np.float16),
        "w2m8": w2m8.astype(NP8),
        "b2mB": b2mB,
        "w1u16": w1u8.astype(np.float16),
        "w2u16": w2u8.astype(np.float16),
        "b1u": np.asarray(inp["upd_b1"], f).reshape(2, 128).T.copy(),
        "b2u": np.asarray(inp["upd_b2"], f).reshape(128, 1).copy(),
        "cw1s": cw1[0:128].astype(np.float16),
        "cw1t": cw1[128:256].astype(np.float16),
        "cb1": np.asarray(inp["cls_b1"], f).reshape(128, 1).copy(),
        "cw2": np.asarray(inp["cls_W2"], f).astype(np.float16),
        "cb2": np.asarray(inp["cls_b2"], f).reshape(20, 1).copy(),
        "id16": np.eye(P, dtype=np.float16),
        "_s0q": s0q.astype(f),
    }


def _host_prep_core(inp, shared, groups):
    """groups: list of 64 lists of 4 global sample indices."""
    f = np.float32
    G = len(groups)
    NQ = G // 4
    src = np.asarray(inp["edge_src"]).astype(np.int64)
    tgt = np.asarray(inp["edge_tgt"]).astype(np.int64)
    rel = np.asarray(inp["edge_rel"]).astype(np.int64)
    ne = np.asarray(inp["n_edges"]).astype(np.int64)
    qs = np.asarray(inp["query_src"]).astype(np.int64)
    qt = np.asarray(inp["query_tgt"]).astype(np.int64)

    oh2 = np.zeros((G, P, 256), np.float16)
    ohe = np.zeros((G, P, P), NP8)
    relq = np.zeros((G // 4, N_REL, 1024), NP8)
    qoh = np.zeros((G, P, 8), np.float16)
    for g, samples in enumerate(groups):
        col = 0
        q, gi = divmod(g, 4)
        for j, sid in enumerate(samples):
            n = int(ne[sid])
            if n > 0:
                ss = src[sid, :n] + 32 * j
                tt = tgt[sid, :n] + 32 * j
                rr = rel[sid, :n]
                cols = np.arange(col, col + n)
                oh2[g, ss, cols] = 1.0
                oh2[g, tt, 128 + cols] = 1.0
                ohe[g, cols, tt] = 1.0
                relq[q, rr, gi * 128 + cols] = 1.0
                relq[q, rr, 512 + gi * 128 + cols] = 1.0
                col += n
            qoh[g, 32 * j + qs[sid], j] = 1.0
            qoh[g, 32 * j + qt[sid], 4 + j] = 1.0

    s0 = np.broadcast_to(shared["_s0q"][None], (NQ, P, 512))
    return {
        "s0": np.ascontiguousarray(s0),
        "u16init": np.ascontiguousarray(s0).astype(np.float16),
        "oh2": oh2,
        "ohe": ohe,
        "relq": relq,
        "qoh16": qoh,
    }


_CACHE = {}


def kernel(**inputs):
    b = np.asarray(inputs["edge_src"]).shape[0]
    b_core = b // N_CORES
    key = b_core
    if key not in _CACHE:
        _CACHE[key] = _build_nc(b_core, N_STEPS)
    nc = _CACHE[key]

    assign = _pack_groups(inputs["n_edges"])
    shared = _host_prep_shared(inputs)
    pub_shared = {k: v for k, v in shared.items() if not k.startswith("_")}
    G_core = b_core // GRP
    in_maps = []
    for c in range(N_CORES):
        groups = assign[c * G_core:(c + 1) * G_core]
        m = dict(pub_shared)
        m.update(_host_prep_core(inputs, shared, groups))
        in_maps.append(m)

    from concourse.bass_utils import run_bass_kernel_spmd
    res = run_bass_kernel_spmd(nc, in_maps, core_ids=list(range(N_CORES)))

    out = np.empty((b, N_REL), dtype=np.float32)
    for c in range(N_CORES):
        oc = res.results[c]["out"]     # (20, b_core)
        groups = assign[c * G_core:(c + 1) * G_core]
        for g, samples in enumerate(groups):
            for j, sid in enumerate(samples):
                out[sid] = oc[:, g * 4 + j]
    return out


# revision 14
# speedup vs baseline: 1.9249x; 1.0015x over previous
"""Trainium2 Bass kernel: CLUTRR-style GNN message passing (nn_CLUTRRV4).

Data-parallel across 8 NeuronCores (256 samples/core). Samples are bin-packed
4-per-group (4 x 32 entity slots = 128 partitions) such that each group's
total valid edge count is <= 128 (greedy + local-search on n_edges), so each
group has a single 128-wide edge block. Gather/scatter are one-hot matmuls
with host-precomputed fp8 one-hots. Per step:

  S8T = transpose(S8)                  # PE transpose, fp8
  xT  = S8T-gather(oh_src|oh_tgt)      # one fp8 matmul, N=256
  h1  = W1[src|tgt]-proj(xT) + reltab  # fp8 DoubleRow + K=20 DR blockdiag
  h1g = gelu(h1) -> fp8
  msg = h1g @ W2m                      # fp8 DoubleRow over hidden halves
  msb = msg + b2m (drain)              # padding edges masked via zero one-hots
  agg = scatter(msb, ohe)              # fp8 matmul
  h3  = W1u^T [S8|agg8]                # fp8 DoubleRow, 2-pair batched N=512
  h3g = gelu(h3 + b1u) -> fp8
  sn  = W2u^T h3g                      # fp8 DoubleRow
  S  += sn + b2u (fp32 master, DVE)    # then refresh fp8 shadow (gpsimd)

Classifier head gathers query states from the final S8T and runs in fp16.
The quantization error of S8 enters h1 only through a K=256 contraction
(project-after-gather), so fp8 errors average out (~1% final).
"""
import sys
import numpy as np

if "/opt/trn_rl_repo" not in sys.path:
    sys.path.append("/opt/trn_rl_repo")

import ml_dtypes

N_ENT, N_REL, D, E_IN = 32, 20, 128, 64
N_STEPS = 8
N_CORES = 8
P = 128
E = 128          # packed edges per group (hard cap, verified by packer)
GRP = 4          # samples per group
NP8 = ml_dtypes.float8_e4m3


def _build_nc(b_core, n_steps):
    from concourse import bacc, mybir
    from concourse.tile import TileContext

    f32 = mybir.dt.float32
    f16 = mybir.dt.float16
    f8 = mybir.dt.float8e4
    AF = mybir.ActivationFunctionType
    OP = mybir.AluOpType
    DRM = mybir.MatmulPerfMode.DoubleRow

    G = b_core // GRP          # 64 groups
    NQ = G // 4                # 16 quads (4 groups = 2 pairs each)

    nc = bacc.Bacc()

    def din(name, shape, dtype=f32):
        return nc.declare_dram_parameter(name, list(shape), dtype, isOutput=False)

    d_s0 = din("s0", (NQ, P, 512))
    d_u8i = din("u16init", (NQ, P, 512), f16)      # initial S16 halves of upd16
    d_oh2 = din("oh2", (G, P, 256), f16)           # [ohs | oht] slot-major
    d_ohe = din("ohe", (G, P, P), f8)              # edge-major tgt one-hot
    d_relq = din("relq", (NQ, N_REL, 1024), f8)    # per-quad rel one-hot [F0 | F1]
    d_rt8 = din("rt8", (N_REL, 256), f8)           # reltab (incl. msg_b1)
    d_w1x = din("w1x16", (P, 512), f16)            # [W1a-F0|W1c-F0|W1a-F1|W1c-F1]
    d_w2m = din("w2m8", (P, 256), f8)              # [W2m rows 0:128 | 128:256]
    d_b2m = din("b2mB", (P, 512))                  # b2m replicated 128p x 4 blocks
    d_w1u = din("w1u16", (P, 512), f16)            # [S|G per mc]
    d_w2u = din("w2u16", (P, 256), f16)
    d_b1u = din("b1u", (P, 2))
    d_b2u = din("b2u", (P, 1))
    d_qoh = din("qoh16", (G, P, 8), f16)
    d_cw1s = din("cw1s", (P, P), f16)
    d_cw1t = din("cw1t", (P, P), f16)
    d_cb1 = din("cb1", (P, 1))
    d_cw2 = din("cw2", (P, N_REL), f16)
    d_cb2 = din("cb2", (N_REL, 1))
    d_id16 = din("id16", (P, P), f16)
    d_out = nc.declare_dram_parameter("out", [N_REL, b_core], f32, isOutput=True)

    with TileContext(nc) as tc:
        with (
            tc.tile_pool(name="c", bufs=1) as cp,
            tc.tile_pool(name="w", bufs=3) as wp,
            tc.tile_pool(name="pT", bufs=1, space="PSUM") as pT,
            tc.tile_pool(name="pX", bufs=2, space="PSUM") as pX,
            tc.tile_pool(name="pH1", bufs=2, space="PSUM") as pH1,
            tc.tile_pool(name="pM", bufs=1, space="PSUM") as pM,
            tc.tile_pool(name="pTl", bufs=2, space="PSUM") as pTl,
        ):
            def cload(name, shape, dram, dtype=f32):
                t = cp.tile(list(shape), dtype, tag=name)
                nc.sync.dma_start(t[:], dram[:])
                return t

            rt8 = cload("rt8", (N_REL, 256), d_rt8, f8)
            w1x = cload("w1x16", (P, 512), d_w1x, f16)
            w2m = cload("w2m8", (P, 256), d_w2m, f8)
            b2m = cload("b2mB", (P, 512), d_b2m)
            w1u = cload("w1u16", (P, 512), d_w1u, f16)
            w2u = cload("w2u16", (P, 256), d_w2u, f16)
            b1u = cload("b1u", (P, 2), d_b1u)
            b2u = cload("b2u", (P, 1), d_b2u)
            cw1s = cload("cw1s", (P, P), d_cw1s, f16)
            cw1t = cload("cw1t", (P, P), d_cw1t, f16)
            cb1 = cload("cb1", (P, 1), d_cb1)
            cw2 = cload("cw2", (P, N_REL), d_cw2, f16)
            cb2 = cload("cb2", (N_REL, 1), d_cb2)
            id16 = cload("id16", (P, P), d_id16, f16)
            outsb = cp.tile([N_REL, b_core], f32, tag="outsb")

            S2, U8 = [], []
            RELQ = [None] * NQ
            OH2, OHE, QOH = [None] * G, [None] * G, [None] * G
            for q in range(NQ):
                RELQ[q] = cp.tile([N_REL, 1024], f8, tag=f"relq_{q}",
                                  name=f"relq_{q}")
                nc.sync.dma_start(RELQ[q][:], d_relq[q])
                t = cp.tile([P, 512], f32, tag=f"S2_{q}")
                nc.sync.dma_start(t[:], d_s0[q])
                S2.append(t)
                t = cp.tile([P, 1024], f16, tag=f"U16_{q}", name=f"U16_{q}")
                nc.sync.dma_start(t[:, 0:512], d_u8i[q])
                U8.append(t)
                for gi in range(4):
                    g = q * 4 + gi
                    OH2[g] = cp.tile([P, 256], f16, tag=f"oh2_{g}",
                                     name=f"oh2_{g}")
                    nc.sync.dma_start(OH2[g][:], d_oh2[g])
                    OHE[g] = cp.tile([P, P], f8, tag=f"ohe_{g}",
                                     name=f"ohe_{g}")
                    nc.sync.dma_start(OHE[g][:], d_ohe[g])

                    QOH[g] = cp.tile([P, 8], f16, tag=f"qoh_{g}",
                                     name=f"qoh_{g}")
                    nc.sync.dma_start(QOH[g][:], d_qoh[g])

            mm = nc.tensor.matmul

            def i2(ap):
                return ap.rearrange("p (i n) -> p i n", i=2)

            def head(q, S16Tsb):
                """Transpose + drain S16T for quad q's 4 groups into S16Tsb."""
                tps = pT.tile([P, 512], f16, tag="tps")
                for gi in range(4):
                    nc.tensor.transpose(
                        tps[:, gi * P:(gi + 1) * P],
                        U8[q][:, gi * P:(gi + 1) * P], id16[:])
                nc.scalar.copy(S16Tsb[:], tps[:])

            for t_step in range(n_steps):
                for q in range(NQ):
                    S16Tsb = wp.tile([P, 512], f16, tag="s16t")
                    head(q, S16Tsb)
                    x16 = wp.tile([P, 1024], f16, tag="x16")
                    for h in range(2):      # two 2-group halves
                        xps = pX.tile([P, 512], f32, tag="xps")
                        for gi in (2 * h, 2 * h + 1):
                            g = q * 4 + gi
                            mm(xps[:, (gi % 2) * 256:(gi % 2) * 256 + 256],
                               lhsT=S16Tsb[:, gi * P:(gi + 1) * P],
                               rhs=OH2[g][:], start=True, stop=True)
                        nc.vector.tensor_copy(x16[:, h * 512:(h + 1) * 512],
                                              xps[:])
                    # h1 per F-chunk, all 4 groups batched (N=512)
                    h1g = wp.tile([P, 1024], f8, tag="h1g")
                    xv = x16[:].rearrange("p (g st e) -> p g st e", st=2, e=P)
                    for F in range(2):
                        h1p = pH1.tile([P, 512], f32, tag="h1p")
                        mm(h1p[:], lhsT=rt8[:, F * P:(F + 1) * P],
                           rhs=RELQ[q][:, F * 512:(F + 1) * 512],
                           start=True, stop=False)
                        for st in range(2):   # 0: src (W1a), 1: tgt (W1c)
                            mm(h1p[:],
                               lhsT=w1x[:, (2 * F + st) * P:(2 * F + st + 1) * P],
                               rhs=xv[:, :, st, :],
                               start=False, stop=(st == 1))
                        nc.scalar.activation(
                            h1g[:, F * 512:(F + 1) * 512], h1p[:], AF.Gelu)
                    # msg layer 2 (edge-major out), 4 groups into one bank
                    mps = pM.tile([P, 512], f32, tag="mps")
                    h1v = h1g[:].rearrange("p (i g e) -> p i g e", i=2, e=P)
                    for gi in range(4):
                        mm(mps[:, gi * P:(gi + 1) * P],
                           lhsT=h1v[:, :, gi, :],
                           rhs=i2(w2m[:]), start=True, stop=True, perf_mode=DRM)
                    msb = wp.tile([P, 512], f8, tag="msb")
                    nc.vector.tensor_tensor(msb[:], mps[:], b2m[:], op=OP.add)
                    # scatter into per-pair agg + update MLP (2-pair batched)
                    agg = pTl.tile([P, 512], f32, tag="tail")
                    for gi in range(4):
                        g = q * 4 + gi
                        mm(agg[:, gi * P:(gi + 1) * P],
                           lhsT=msb[:, gi * P:(gi + 1) * P],
                           rhs=OHE[g][:], start=True, stop=True)
                    nc.vector.tensor_copy(U8[q][:, 512:1024], agg[:])
                    h3g = wp.tile([P, 1024], f16, tag="h3g")
                    for mc in range(2):
                        h3p = pTl.tile([P, 512], f32, tag="tail")
                        mm(h3p[:], lhsT=w1u[:, mc * 256:mc * 256 + P],
                           rhs=U8[q][:, 0:512], start=True, stop=False)
                        mm(h3p[:], lhsT=w1u[:, mc * 256 + P:(mc + 1) * 256],
                           rhs=U8[q][:, 512:1024], start=False, stop=True)
                        nc.scalar.activation(
                            h3g[:, mc * 512:(mc + 1) * 512], h3p[:], AF.Gelu,
                            bias=b1u[:, mc:mc + 1])
                    snp = pTl.tile([P, 512], f32, tag="tail")
                    for mc in range(2):
                        mm(snp[:], lhsT=w2u[:, mc * P:(mc + 1) * P],
                           rhs=h3g[:, mc * 512:(mc + 1) * 512],
                           start=(mc == 0), stop=(mc == 1))
                    nc.vector.scalar_tensor_tensor(
                        out=S2[q][:], in0=snp[:], scalar=b2u[:, 0:1],
                        in1=S2[q][:], op0=OP.add, op1=OP.add)
                    nc.gpsimd.tensor_copy(U8[q][:, 0:512], S2[q][:])

            # classifier head: gather query states from final S8T, fp16 MLP
            for q in range(NQ):
                S16Tsb = wp.tile([P, 512], f16, tag="s16t")
                head(q, S16Tsb)
                qps = pX.tile([P, 512], f32, tag="xps")
                for gi in range(4):
                    g = q * 4 + gi
                    mm(qps[:, gi * 8:(gi + 1) * 8],
                       lhsT=S16Tsb[:, gi * P:(gi + 1) * P],
                       rhs=QOH[g][:], start=True, stop=True)
                qt16 = wp.tile([P, 32], f16, tag="qt16")
                nc.vector.tensor_copy(qt16[:], qps[:, 0:32])
                qv = qt16[:].rearrange("p (g t f) -> p g t f", t=2, f=4)
                hcp = pH1.tile([P, 512], f32, tag="h1p")
                mm(hcp[:, 0:16], lhsT=cw1s[:], rhs=qv[:, :, 0, :],
                   start=True, stop=False)
                mm(hcp[:, 0:16], lhsT=cw1t[:], rhs=qv[:, :, 1, :],
                   start=False, stop=True)
                hcg = wp.tile([P, 16], f16, tag="hcg")
                nc.scalar.activation(hcg[:], hcp[:, 0:16], AF.Gelu,
                                     bias=cb1[:, 0:1])
                lps = pM.tile([P, 512], f32, tag="mps")
                mm(lps[0:N_REL, 0:16], lhsT=cw2[:], rhs=hcg[:],
                   start=True, stop=True)
                nc.scalar.activation(
                    outsb[:, q * 16:(q + 1) * 16], lps[0:N_REL, 0:16],
                    AF.Identity, bias=cb2[:, 0:1])
            nc.sync.dma_start(d_out[:], outsb[:])

    nc.finalize()
    return nc


def _pack_groups(n_edges):
    """Pack 2048 samples into 512 groups of 4 with per-group edge sum <= 128.
    Returns list of 512 lists of 4 sample indices. Falls back to max found."""
    import heapq, random
    ne = np.asarray(n_edges, dtype=np.int64)
    B = ne.shape[0]
    G = B // GRP
    order = np.argsort(-ne)
    gsum = np.zeros(G, np.int64)
    gcnt = np.zeros(G, np.int64)
    assign = [[] for _ in range(G)]
    heap = [(0, 0, g) for g in range(G)]
    heapq.heapify(heap)
    for idx in order:
        while True:
            s, c, g = heapq.heappop(heap)
            if gcnt[g] == c and gsum[g] == s:
                break
        assign[g].append(int(idx))
        gsum[g] += ne[idx]
        gcnt[g] += 1
        if gcnt[g] < GRP:
            heapq.heappush(heap, (gsum[g], gcnt[g], g))
    rng = random.Random(0)
    for _ in range(300000):
        over = [g for g in range(G) if gsum[g] > E]
        if not over:
            break
        g1 = rng.choice(over)
        found = False
        idxs1 = sorted(assign[g1], key=lambda i: -ne[i])
        g2s = list(range(G))
        rng.shuffle(g2s)
        for a in idxs1:
            na = ne[a]
            for g2 in g2s:
                if g2 == g1:
                    continue
                for b in assign[g2]:
                    nb = ne[b]
                    if nb < na and gsum[g2] + na - nb <= E and \
                            gsum[g1] - na + nb < gsum[g1]:
                        assign[g1].remove(a); assign[g2].remove(b)
                        assign[g1].append(b); assign[g2].append(a)
                        gsum[g1] += nb - na; gsum[g2] += na - nb
                        found = True
                        break
                if found:
                    break
            if found:
                break
        if not found:
            for _ in range(4000):
                g2 = rng.randrange(G)
                if g2 == g1:
                    continue
                a = rng.choice(assign[g1]); b = rng.choice(assign[g2])
                na, nb = ne[a], ne[b]
                n1 = gsum[g1] - na + nb
                n2 = gsum[g2] - nb + na
                if n1 <= gsum[g1] and n2 <= max(E, gsum[g2]) and \
                        not (n1 == gsum[g1] and n2 == gsum[g2]):
                    assign[g1].remove(a); assign[g2].remove(b)
                    assign[g1].append(b); assign[g2].append(a)
                    gsum[g1] = n1; gsum[g2] = n2
                    break
    assert gsum.max() <= E, f"edge packing failed: max group sum {gsum.max()}"
    return assign


def _host_prep_shared(inp):
    f = np.float32
    w1 = np.asarray(inp["msg_W1"], f)
    reltab = np.asarray(inp["rel_embed"], f) @ w1[128:256] + \
        np.asarray(inp["msg_b1"], f)                      # (20, 256)
    W1a, W1c = w1[0:128], w1[256:384]                     # (128, 256) each
    w1x = np.concatenate(
        [W1a[:, 0:128], W1c[:, 0:128], W1a[:, 128:256], W1c[:, 128:256]],
        axis=1)                                           # (128, 512)
    w2m_ = np.asarray(inp["msg_W2"], f)                   # (256, 128)
    w2m8 = np.concatenate([w2m_[0:128], w2m_[128:256]], axis=1)  # (128, 256)
    b2m = np.asarray(inp["msg_b2"], f)                    # (128,)
    b2mB = np.tile(b2m[None, :], (P, 4)).astype(f)        # (128, 512)
    u1 = np.asarray(inp["upd_W1"], f)                     # (256, 256)
    w1u8 = np.concatenate(
        [u1[0:128, 0:128], u1[128:256, 0:128],
         u1[0:128, 128:256], u1[128:256, 128:256]], axis=1)  # (128, 512)
    u2 = np.asarray(inp["upd_W2"], f)                     # (256, 128)
    w2u8 = np.concatenate([u2[0:128], u2[128:256]], axis=1)
    cw1 = np.asarray(inp["cls_W1"], f)                    # (256, 128)
    ee = np.asarray(inp["entity_embed"], f)               # (32, 128)
    s0q = np.tile(ee.T, (1, 16))                          # (128, 512) per quad
    return {
        "rt8": reltab.astype(NP8),
        "w1x16": w1x.astype(np.float16),
        "w2m8": w2m8.astype(NP8),
        "b2mB": b2mB,
        "w1u16": w1u8.astype(np.float16),
        "w2u16": w2u8.astype(np.float16),
        "b1u": np.asarray(inp["upd_b1"], f).reshape(2, 128).T.copy(),
        "b2u": np.asarray(inp["upd_b2"], f).reshape(128, 1).copy(),
        "cw1s": cw1[0:128].astype(np.float16),
        "cw1t": cw1[128:256].astype(np.float16),
        "cb1": np.asarray(inp["cls_b1"], f).reshape(128, 1).copy(),
        "cw2": np.asarray(inp["cls_W2"], f).astype(np.float16),
        "cb2": np.asarray(inp["cls_b2"], f).reshape(20, 1).copy(),
        "id16": np.eye(P, dtype=np.float16),
        "_s0q": s0q.astype(f),
    }


def _host_prep_core(inp, shared, groups):
    """groups: list of 64 lists of 4 global sample indices."""
    f = np.float32
    G = len(groups)
    NQ = G // 4
    src = np.asarray(inp["edge_src"]).astype(np.int64)
    tgt = np.asarray(inp["edge_tgt"]).astype(np.int64)
    rel = np.asarray(inp["edge_rel"]).astype(np.int64)
    ne = np.asarray(inp["n_edges"]).astype(np.int64)
    qs = np.asarray(inp["query_src"]).astype(np.int64)
    qt = np.asarray(inp["query_tgt"]).astype(np.int64)

    oh2 = np.zeros((G, P, 256), np.float16)
    ohe = np.zeros((G, P, P), NP8)
    relq = np.zeros((G // 4, N_REL, 1024), NP8)
    qoh = np.zeros((G, P, 8), np.float16)
    for g, samples in enumerate(groups):
        col = 0
        q, gi = divmod(g, 4)
        for j, sid in enumerate(samples):
            n = int(ne[sid])
            if n > 0:
                ss = src[sid, :n] + 32 * j
                tt = tgt[sid, :n] + 32 * j
                rr = rel[sid, :n]
                cols = np.arange(col, col + n)
                oh2[g, ss, cols] = 1.0
                oh2[g, tt, 128 + cols] = 1.0
                ohe[g, cols, tt] = 1.0
                relq[q, rr, gi * 128 + cols] = 1.0
                relq[q, rr, 512 + gi * 128 + cols] = 1.0
                col += n
            qoh[g, 32 * j + qs[sid], j] = 1.0
            qoh[g, 32 * j + qt[sid], 4 + j] = 1.0

    s0 = np.broadcast_to(shared["_s0q"][None], (NQ, P, 512))
    return {
        "s0": np.ascontiguousarray(s0),
        "u16init": np.ascontiguousarray(s0).astype(np.float16),
        "oh2": oh2,
        "ohe": ohe,
        "relq": relq,
        "qoh16": qoh,
    }


_CACHE = {}


def _patch_ldw_opt():
    from concourse import bass_utils as bu
    if getattr(bu, "_ldw_opt_patched", False):
        return
    orig = bu.run_command

    def run_command_ldw(cmd, *a, **kw):
        if isinstance(cmd, list):
            cmd = [c.replace("--enable-ldw-opt=false", "--enable-ldw-opt=true")
                   if isinstance(c, str) else c for c in cmd]
        return orig(cmd, *a, **kw)

    bu.run_command = run_command_ldw
    bu._ldw_opt_patched = True


def kernel(**inputs):
    b = np.asarray(inputs["edge_src"]).shape[0]
    b_core = b // N_CORES
    key = b_core
    if key not in _CACHE:
        _CACHE[key] = _build_nc(b_core, N_STEPS)
    nc = _CACHE[key]

    assign = _pack_groups(inputs["n_edges"])
    shared = _host_prep_shared(inputs)
    pub_shared = {k: v for k, v in shared.items() if not k.startswith("_")}
    G_core = b_core // GRP
    in_maps = []
    for c in range(N_CORES):
        groups = assign[c * G_core:(c + 1) * G_core]
        m = dict(pub_shared)
        m.update(_host_prep_core(inputs, shared, groups))
        in_maps.append(m)

    from concourse.bass_utils import run_bass_kernel_spmd
    res = run_bass_kernel_spmd(nc, in_maps, core_ids=list(range(N_CORES)))

    out = np.empty((b, N_REL), dtype=np.float32)
    for c in range(N_CORES):
        oc = res.results[c]["out"]     # (20, b_core)
        groups = assign[c * G_core:(c + 1) * G_core]
        for g, samples in enumerate(groups):
            for j, sid in enumerate(samples):
                out[sid] = oc[:, g * 4 + j]
    return out
